# revision 2
# baseline (speedup 1.0000x reference)
"""ClsAttention pooling kernel for 8 TRN2 NeuronCores.

reference:
    att_logits = einsum('bch,nc->bnh', feats, W)      # [B, N, HW]
    att_maps   = softmax(att_logits, axis=2)          # softmax over HW
    cls_feats  = einsum('bnh,bch->bnc', att_maps, feats)

Strategy (data-parallel over batch, 4 items per core, per-block pipeline):
  - One HBM pass over feats (33.5MB f32/core) = the hard floor (~85us of DMA).
  - Loads are per-h-block (512h) SWDGE cast-DMAs (f32->fp16) so mm1 can start
    as soon as the first ~0.5MB lands instead of waiting for a 4MB slab.
    wt is issued first (tiny), identity-gen after the first fb loads.
  - Per 512-block: mm1 (W-stationary, 4 c-chunks -> PSUM [80,512]) -> exp on
    ScalarE (accum_out -> softmax partial Z) -> f^T on PE transpose + DVE
    drains -> mm2 (U += eT^T @ fT per 128h chunk, PSUM held across the item).
  - eT+mm2 of block cb are emitted after fT of block cb+1 so the PE never
    stalls on ScalarE's exp latency.
  - First item starts 256+256 blocks (earlier PE start); last item ends
    384+128 (shorter post-last-byte drain chain).
"""

import numpy as np

import concourse.bass as bass
import concourse.mybir as mybir
import concourse.tile as tile
from concourse import bacc
from concourse.bass_utils import run_bass_kernel_spmd
from concourse.masks import make_identity

B, C, HW, NCLS = 32, 512, 4096, 80
NCORES = 8
BPC = B // NCORES   # batch items per core
CCH = C // 128      # c chunks (mm1/mm2 layout)
FB_BUFS = 12        # fb (cast load) buffers; deep so the DMA streams ahead
FT_BUFS = 3         # transposed-feats buffers
E_BUFS = 3
ET_BUFS = 3
PLP_BUFS = 3        # mm1 logits PSUM banks
PTP_BUFS = 3        # transpose-drain PSUM banks
STORE_ENG = "sync"  # HWDGE ring for output stores
CDT = mybir.dt.float16
F32 = mybir.dt.float32

# per-item h-block plans (sum = HW)
_PLANS = [
    [256, 256] + [512] * 7,   # item 0: small lead-in blocks -> early PE start
    [512] * 8,
    [512] * 8,
    [512] * 7 + [384, 128],   # last item: tapered tail -> short drain chain
]

_cached_nc = None


def _build():
    global _cached_nc
    if _cached_nc is not None:
        return _cached_nc
    nc = bacc.Bacc("TRN2", target_bir_lowering=False, debug=False)
    feats = nc.dram_tensor("feats", [BPC, C, HW], F32, kind="ExternalInput")
    wt = nc.dram_tensor("wt", [C, NCLS], F32, kind="ExternalInput")
    out = nc.dram_tensor("out", [BPC, NCLS, C], F32, kind="ExternalOutput")
    fview = feats.rearrange("b (ci p) h -> b p ci h", p=128)

    # flat load schedule: (item, block_idx, h0, hw)
    sched = []
    for b in range(BPC):
        h0 = 0
        for cb, hw in enumerate(_PLANS[b]):
            sched.append((b, cb, h0, hw))
            h0 += hw

    with tile.TileContext(nc) as tc:
        with (
            tc.tile_pool(name="singles", bufs=1) as singles,
            tc.tile_pool(name="fpool", bufs=FB_BUFS) as fpool,
            tc.tile_pool(name="tpool", bufs=FT_BUFS) as tpool,
            tc.tile_pool(name="epool", bufs=E_BUFS) as epool,
            tc.tile_pool(name="etpool", bufs=ET_BUFS) as etpool,
            tc.tile_pool(name="zpool", bufs=2) as zpool,
            tc.tile_pool(name="opool", bufs=2) as opool,
            tc.tile_pool(name="plp", bufs=PLP_BUFS, space="PSUM") as plp,
            tc.tile_pool(name="pup", bufs=2, space="PSUM") as pup,
            tc.tile_pool(name="ptp", bufs=PTP_BUFS, space="PSUM") as ptp,
        ):
            # weights first (small), then the first fb blocks, THEN identity
            # gen, so the PE-critical loads hit the SWDGE queue first.
            wt_sb = singles.tile([128, CCH, NCLS], CDT)
            nc.gpsimd.dma_start(
                out=wt_sb, in_=wt.rearrange("(ci p) n -> p ci n", p=128)
            )

            fbs = [None] * len(sched)

            def emit_load(i):
                b, cb, h0, hw = sched[i]
                fb = fpool.tile([128, CCH, hw], CDT, name="fb", tag="fb")
                nc.gpsimd.dma_start(out=fb, in_=fview[b, :, :, h0 : h0 + hw])
                fbs[i] = fb

            emit_load(0)
            emit_load(1)

            ident = singles.tile([128, 128], CDT)
            make_identity(nc, ident)

            for i in range(2, len(sched)):
                emit_load(i)

            li = 0  # index into sched/fbs
            for b in range(BPC):
                plan = _PLANS[b]
                nblk = len(plan)
                zp = zpool.tile([NCLS, nblk], F32)
                pu = pup.tile([NCLS, C], F32)

                pend = None  # (E_blk, ftT, nch, first)

                def flush(last):
                    E_blk, ftT, nch, first = pend
                    pe_ = ptp.tile([128, nch, NCLS], CDT, name="pe_", tag="pt")
                    for t in range(nch):
                        nc.tensor.transpose(
                            pe_[:, t, :],
                            E_blk[:, bass.ts(t, 128)],
                            ident[0:NCLS, 0:NCLS],
                        )
                    eT = etpool.tile([128, nch, NCLS], CDT, name="eT", tag="eT")
                    nc.vector.tensor_copy(out=eT, in_=pe_)
                    for hj in range(nch):
                        nc.tensor.matmul(
                            pu,
                            lhsT=eT[:, hj, :],
                            rhs=ftT[:, :, hj, :],
                            start=(first and hj == 0),
                            stop=(last and hj == nch - 1),
                        )

                for cb, hw in enumerate(plan):
                    fb = fbs[li]
                    li += 1
                    nch = hw // 128
                    # mm1 (weight stationary) + exp + Z accumulation
                    pl = plp.tile([NCLS, hw], F32, name="pl", tag="pl")
                    for ci in range(CCH):
                        nc.tensor.matmul(
                            pl,
                            lhsT=wt_sb[:, ci, :],
                            rhs=fb[:, ci, :],
                            start=(ci == 0),
                            stop=(ci == CCH - 1),
                        )
                    E_blk = epool.tile([NCLS, hw], CDT, name="E", tag="E")
                    nc.scalar.activation(
                        out=E_blk,
                        in_=pl,
                        func=mybir.ActivationFunctionType.Exp,
                        accum_out=zp[:, cb : cb + 1],
                    )
                    # feats^T on PE: ftT[p, ci, t, c] = fb[c, t*128+p]
                    ftT = tpool.tile([128, CCH, nch, 128], CDT, name="ftT", tag="ftT")
                    for ci in range(CCH):
                        pt = ptp.tile([128, nch, 128], CDT, name="pt", tag="pt")
                        for t in range(nch):
                            nc.tensor.transpose(
                                pt[:, t, :],
                                fb[:, ci, bass.ts(t, 128)],
                                ident,
                            )
                        nc.vector.tensor_copy(out=ftT[:, ci], in_=pt)
                    # previous block's eT + mm2 (hides exp latency behind fT)
                    if pend is not None:
                        flush(False)
                    pend = (E_blk, ftT, nch, cb == 0)
                flush(True)
                pend = None

                # Z, 1/Z, cls = U / Z
                z = zpool.tile([NCLS, 1], F32, name="z", tag="z")
                nc.vector.reduce_sum(z, zp, axis=mybir.AxisListType.X)
                zr = zpool.tile([NCLS, 1], F32, name="zr", tag="z")
                nc.vector.reciprocal(zr, z)
                ob = opool.tile([NCLS, C], F32)
                nc.vector.tensor_scalar_mul(ob, pu, zr)
                getattr(nc, STORE_ENG).dma_start(out=out[b], in_=ob)

    nc.compile()
    _cached_nc = nc
    return nc


def kernel(feats: np.ndarray, W: np.ndarray, **run_kwargs) -> np.ndarray:
    nc = _build()
    feats = np.ascontiguousarray(np.asarray(feats), dtype=np.float32)
    wt = np.ascontiguousarray(np.asarray(W, dtype=np.float32).T)
    in_maps = [
        {"feats": np.ascontiguousarray(feats[i * BPC : (i + 1) * BPC]), "wt": wt}
        for i in range(NCORES)
    ]
    res = run_bass_kernel_spmd(nc, in_maps, list(range(NCORES)), **run_kwargs)
    out = np.concatenate([r["out"] for r in res.results], axis=0)
    if run_kwargs:
        kernel.last_results = res
    return np.asarray(out, dtype=np.float32)
